# revision 5
# baseline (speedup 1.0000x reference)
"""Multi-head attention block (B=2, N=2048, C=1024, H=16, D=64) on 8
Trainium2 NeuronCores — fused-pipeline version.

Sharding: core c -> batch b = c//4, head-group g = c%4 (tensor-parallel over
heads within a batch, 4 heads per core). QKV weights column-sharded, w_proj
row-sharded; each core emits a partial [N, C] projection which the host sums
per batch and then adds b_proj.

Final version (v4a):
- bf16 pipeline end to end (inputs pre-cast on host), tol 2e-2 passes at ~3e-3.
- Single fused emission: K(all) -> V(all) -> Q(0), then per-q-slab attention
  with Q(s+1)/proj(s-1) units interleaved between head-pairs so ACT exp and
  DVE work overlap the QKV/proj matmuls.
- Head-pair QK^T row-packing: the two heads of a pair sit at partitions
  0-63 / 64-127, so their K=64-contraction QK^T matmuls carry disjoint PE
  row-groups (tile_position auto-derived from base_partition) and execute
  concurrently on the 128x128 array; one exp instruction covers both heads'
  S^T chunk ([128, 1024]); PV lags one chunk.
- x shipped host-swizzled as [128, slab, kc, 512] so each per-slab DMA is a
  contiguous 8KB run per partition.
- PSUM: 2 S^T rings (2 banks each) + 2 po + 2 shared qkv/proj banks.
"""

import sys

sys.path.insert(0, "/opt/trn_rl_repo")

import numpy as np

from contextlib import ExitStack

import concourse.bacc as bacc
import concourse.tile as tile
from concourse import mybir

F32 = mybir.dt.float32
BF16 = mybir.dt.bfloat16

N = 2048
C = 1024
HL = 4  # heads per core
D = 64
KC = C // 128  # 8 contraction chunks
NS = N // 512  # 4 n-supers / slabs
MC = N // 128  # 16 m-chunks


def build_attention_nc(mm_dtype=BF16, loop_iters=None, stag=False, nring=2, glen=2, probes=frozenset()):
    nc = bacc.Bacc(None, target_bir_lowering=False, debug=False)

    MMDT = mm_dtype
    xt = nc.dram_tensor("xt", [128, NS, KC, 512], MMDT, kind="ExternalInput")
    wq = nc.dram_tensor("wq", [C, 2, 128], MMDT, kind="ExternalInput")
    wk = nc.dram_tensor("wk", [C, 2, 128], MMDT, kind="ExternalInput")
    wv = nc.dram_tensor("wv", [C, 256], MMDT, kind="ExternalInput")
    bq = nc.dram_tensor("bq", [2, 128], F32, kind="ExternalInput")
    bk = nc.dram_tensor("bk", [2, 128], F32, kind="ExternalInput")
    bv = nc.dram_tensor("bv", [256], F32, kind="ExternalInput")
    wp = nc.dram_tensor("wp", [256, C], MMDT, kind="ExternalInput")
    onesv = nc.dram_tensor("onesv", [128, 64], MMDT, kind="ExternalInput")
    out = nc.dram_tensor("out", [N, C], MMDT, kind="ExternalOutput")

    with (
        tile.TileContext(nc) as tc,
        ExitStack() as ctx,
        nc.allow_low_precision(reason="bf16 matmul pipeline, tol 2e-2"),
    ):
        const = ctx.enter_context(tc.tile_pool(name="const", bufs=1))
        persist = ctx.enter_context(tc.tile_pool(name="persist", bufs=1))
        xt_pool = ctx.enter_context(tc.tile_pool(name="xt_pool", bufs=1))
        mm_ps = ctx.enter_context(tc.tile_pool(name="mm_ps", bufs=2, space="PSUM"))
        st_ps = ctx.enter_context(tc.tile_pool(name="st_ps", bufs=1, space="PSUM"))
        o_ps = ctx.enter_context(tc.tile_pool(name="o_ps", bufs=1, space="PSUM"))
        p_pool = ctx.enter_context(tc.tile_pool(name="p_pool", bufs=4))
        r_pool = ctx.enter_context(tc.tile_pool(name="r_pool", bufs=4))
        out_pool = ctx.enter_context(tc.tile_pool(name="out_pool", bufs=3))

        # --- constants / weights (emission order = DMA priority) ---
        wk_sb = const.tile([128, KC, 256], MMDT)
        wv_sb = const.tile([128, KC, 256], MMDT)
        wq_sb = const.tile([128, KC, 256], MMDT)
        nc.sync.dma_start(out=wk_sb, in_=wk.rearrange("(kc p) j m -> p kc (j m)", p=128))
        bk_sb = const.tile([128, 2], F32)
        nc.sync.dma_start(out=bk_sb, in_=bk.rearrange("j p -> p j"))
        nc.sync.dma_start(out=wv_sb, in_=wv.rearrange("(kc p) m -> p kc m", p=128))
        bv_rep = const.tile([128, 256], F32)
        nc.sync.dma_start(out=bv_rep, in_=bv[:].unsqueeze(0).partition_broadcast(128))
        nc.sync.dma_start(out=wq_sb, in_=wq.rearrange("(kc p) j m -> p kc (j m)", p=128))
        bq_sb = const.tile([128, 2], F32)
        nc.sync.dma_start(out=bq_sb, in_=bq.rearrange("j p -> p j"))
        wp_sb = const.tile([128, 2, C], MMDT)
        nc.sync.dma_start(out=wp_sb, in_=wp.rearrange("(kc p) n -> p kc n", p=128))

        # --- persistent intermediates ---
        qt_sb = persist.tile([128, 2, N], MMDT)  # [d(2 heads), pair, n]
        kt_sb = persist.tile([128, 2, N], MMDT)
        v_sb = persist.tile([128, MC, HL, 65], MMDT)  # [m, mc, head, d|1]
        ot_sb = persist.tile([128, 2, N], MMDT)  # [d(2 heads), hd-chunk, n]
        nc.sync.dma_start(
            out=v_sb[:, :, :, 64:65],
            in_=onesv.rearrange("p (a b c) -> p a b c", a=MC, b=HL, c=1),
        )

        def body():
            # x resident for all slabs, loaded per-slab for fine-grain deps
            xts = xt_pool.tile([128, NS, KC, 512], MMDT, tag="xts")
            if "xdma" not in probes:
                for s in range(NS):
                    nc.sync.dma_start(
                        out=xts[:, s, :, :],
                        in_=xt[:, s, :, :],
                    )

            def emit_qk(s, j, wsb, bsb, dst, nm):
                if "qkv" in probes:
                    return
                ps = mm_ps.tile([128, 512], F32, tag="mmps", name=f"{nm}{s}{j}")
                for kc in range(KC):
                    nc.tensor.matmul(
                        ps,
                        lhsT=wsb[:, kc, j * 128 : (j + 1) * 128],
                        rhs=xts[:, s, kc, :],
                        start=kc == 0,
                        stop=kc == KC - 1,
                    )
                nc.vector.tensor_scalar_add(
                    out=dst[:, j, s * 512 : (s + 1) * 512],
                    in0=ps,
                    scalar1=bsb[:, j : j + 1],
                )

            def emit_v(s, jj):
                if "qkv" in probes:
                    return
                ps = mm_ps.tile([128, 256], F32, tag="mmps", name=f"v{s}{jj}")
                for kc in range(KC):
                    nc.tensor.matmul(
                        ps,
                        lhsT=xts[:, s, kc, jj * 128 : (jj + 1) * 128],
                        rhs=wv_sb[:, kc, :],
                        start=kc == 0,
                        stop=kc == KC - 1,
                    )
                mc = s * 4 + jj
                nc.vector.tensor_add(
                    out=v_sb[:, mc, :, 0:64],
                    in0=ps.rearrange("p (h d) -> p h d", h=HL),
                    in1=bv_rep.rearrange("p (h d) -> p h d", h=HL),
                )

            def emit_proj(s, ntl):
                if "proj" in probes:
                    return
                nt = s * 4 + ntl
                for cc in range(2):
                    ps = mm_ps.tile([128, 512], F32, tag="mmps", name=f"pj{nt}{cc}")
                    for hdc in range(2):
                        nc.tensor.matmul(
                            ps,
                            lhsT=ot_sb[:, hdc, nt * 128 : (nt + 1) * 128],
                            rhs=wp_sb[:, hdc, cc * 512 : (cc + 1) * 512],
                            start=hdc == 0,
                            stop=hdc == 1,
                        )
                    so = out_pool.tile([128, 512], MMDT, tag="so")
                    nc.vector.tensor_copy(out=so, in_=ps)
                    nc.sync.dma_start(
                        out=out[nt * 128 : (nt + 1) * 128, cc * 512 : (cc + 1) * 512],
                        in_=so,
                    )

            # ---- prologue: K for all slabs (every m-chunk of pair (0,0)
            # reads K), V(0) (PV chunks 0-3), Q(0). V(1..3) are emitted as
            # per-chunk units inside pair (0,0) just ahead of their PV use;
            # Q(s+1)/proj(s-1) spread over later pairs. Keeping the prologue
            # minimal gets the first exp onto ACT (the serial bottleneck)
            # ~14 us earlier than emitting all of QKV up front. ----
            for s in range(NS):
                for j in range(2):
                    emit_qk(s, j, wk_sb, bk_sb, kt_sb, "k")
            for jj in range(4):
                emit_v(0, jj)
            for j in range(2):
                emit_qk(0, j, wq_sb, bq_sb, qt_sb, "q")

            # per-(s,j) units keyed by chunk slot: emitted between exp(mc)
            # and PV(mc-1) so the next QK^T is never delayed by more than
            # one unit (ACT stays fed).
            def qk_unit(s2, j2):
                return lambda: emit_qk(s2, j2, wq_sb, bq_sb, qt_sb, "q")

            U = {
                (0, 0): {
                    slot: (lambda vs=1 + slot // 4, jj=slot % 4: emit_v(vs, jj))
                    for slot in range(12)
                },
                (0, 1): {2: qk_unit(1, 0), 8: qk_unit(1, 1)},
                (1, 0): {4 * n + 2: (lambda n=n: emit_proj(0, n)) for n in range(4)},
                (1, 1): {2: qk_unit(2, 0), 8: qk_unit(2, 1)},
                (2, 0): {4 * n + 2: (lambda n=n: emit_proj(1, n)) for n in range(4)},
                (2, 1): {2: qk_unit(3, 0), 8: qk_unit(3, 1)},
                (3, 0): {4 * n + 2: (lambda n=n: emit_proj(2, n)) for n in range(4)},
                (3, 1): {},
            }

            # rings hold one m-chunk for BOTH heads of a pair: [128, hh, 512]
            rings = [
                st_ps.tile([128, 2, 512], F32, tag=f"ring{u}", name=f"ring{u}")
                for u in range(nring)
            ]

            for s in range(NS):
                for j in range(2):
                    units = U[(s, j)]
                    po2 = [
                        o_ps.tile([128, 512], F32, tag=f"po{hh}", name=f"po_s{s}j{j}h{hh}")
                        for hh in range(2)
                    ]

                    def emit_pv(prev):
                        if "pv" in probes:
                            return
                        pt, mc = prev
                        for hh in range(2):
                            nc.tensor.matmul(
                                po2[hh][0:65, :],
                                lhsT=v_sb[:, mc, 2 * j + hh, :],
                                rhs=pt[:, hh, :],
                                start=mc == 0,
                                stop=mc == MC - 1,
                            )

                    prev = None
                    for mc in range(MC):
                        ring = rings[mc % nring]
                        # both heads' QK^T back-to-back: base partitions 0/64
                        # -> disjoint PE row groups, run concurrently
                        for hh in range(2 if "qkt" not in probes else 0):
                            base = hh * 64
                            nc.tensor.matmul(
                                ring[:, hh, :],
                                lhsT=kt_sb[
                                    base : base + 64, j, mc * 128 : (mc + 1) * 128
                                ],
                                rhs=qt_sb[
                                    base : base + 64, j, s * 512 : (s + 1) * 512
                                ],
                                start=True,
                                stop=True,
                            )
                        pt = p_pool.tile([128, 2, 512], MMDT, tag="pt")
                        if "exp" not in probes:
                            nc.scalar.activation(
                                out=pt,
                                in_=ring,
                                func=mybir.ActivationFunctionType.Exp,
                            )
                        if mc in units:
                            units[mc]()
                        if prev is not None:
                            emit_pv(prev)
                        prev = (pt, mc)
                    emit_pv(prev)

                    for hh in range(2):
                        recip = r_pool.tile([1, 512], F32, tag="recip")
                        nc.vector.reciprocal(out=recip, in_=po2[hh][64:65, :])
                        rden_sb = r_pool.tile(
                            [64, 512], F32, tag="rden_sb", name=f"rd_s{s}j{j}h{hh}"
                        )
                        nc.gpsimd.partition_broadcast(rden_sb, recip)
                        nc.vector.tensor_mul(
                            out=ot_sb[
                                hh * 64 : hh * 64 + 64,
                                j,
                                s * 512 : (s + 1) * 512,
                            ],
                            in0=po2[hh][0:64, :],
                            in1=rden_sb,
                        )

            # epilogue: proj of last slab
            for n in range(4):
                emit_proj(3, n)

        if loop_iters is None:
            body()
        else:
            with tc.For_i(0, loop_iters, 1, staggered_reset=stag):
                body()

    nc.compile()
    return nc


EMBED_DIM = 1024
NUM_HEADS = 16
HEAD_DIM = 64
HPC = 4

_CACHE = {}


def _bf16(a):
    import ml_dtypes

    return np.asarray(a, dtype=ml_dtypes.bfloat16)


def _make_in_maps(x, w_qkv, b_qkv, w_proj):
    scale = HEAD_DIM ** -0.5
    def _swizzle(xb):
        xt = np.ascontiguousarray(xb.T)  # [C, N]
        return _bf16(
            np.ascontiguousarray(
                xt.reshape(KC, 128, NS, 512).transpose(1, 2, 0, 3)
            )
        )

    xts = [_swizzle(x[b]) for b in range(2)]
    ones = _bf16(np.ones((128, 64), np.float32))
    in_maps = []
    for core in range(8):
        b, g = core // 4, core % 4
        cols = slice(g * HPC * HEAD_DIM, (g + 1) * HPC * HEAD_DIM)
        wq = (w_qkv[:, 0:C][:, cols] * scale).astype(np.float32)
        wk = w_qkv[:, C : 2 * C][:, cols].astype(np.float32)
        wv = w_qkv[:, 2 * C : 3 * C][:, cols].astype(np.float32)
        bq = (b_qkv[0:C][cols] * scale).astype(np.float32)
        bk = b_qkv[C : 2 * C][cols].astype(np.float32)
        bvv = b_qkv[2 * C : 3 * C][cols].astype(np.float32)
        wpm = np.ascontiguousarray(w_proj[cols.start : cols.stop, :]).astype(np.float32)
        in_maps.append(
            {
                "xt": xts[b],
                "wq": _bf16(np.ascontiguousarray(wq.reshape(C, 2, 128))),
                "wk": _bf16(np.ascontiguousarray(wk.reshape(C, 2, 128))),
                "wv": _bf16(np.ascontiguousarray(wv)),
                "bq": np.ascontiguousarray(bq.reshape(2, 128)),
                "bk": np.ascontiguousarray(bk.reshape(2, 128)),
                "bv": np.ascontiguousarray(bvv),
                "wp": _bf16(wpm),
                "onesv": ones,
            }
        )
    return in_maps


def kernel(x, w_qkv, b_qkv, w_proj, b_proj):
    from concourse.bass_utils import run_bass_kernel_spmd

    x = np.asarray(x)
    w_qkv = np.asarray(w_qkv)
    b_qkv = np.asarray(b_qkv)
    w_proj = np.asarray(w_proj)
    b_proj = np.asarray(b_proj)

    if "nc" not in _CACHE:
        _CACHE["nc"] = build_attention_nc()
    nc = _CACHE["nc"]

    in_maps = _make_in_maps(x, w_qkv, b_qkv, w_proj)
    # The device transiently wedges on some runs (INTERNAL error on a NEFF
    # that runs clean on retry). Retry, then rebuild once, before giving up.
    import os

    res = None
    for attempt in range(3):
        try:
            res = run_bass_kernel_spmd(nc, in_maps, core_ids=list(range(8)))
            break
        except Exception:
            if attempt == 2:
                raise
            os.environ.setdefault("NEURON_RT_RESET_CORES", "1")
            if attempt == 1:
                _CACHE.pop("nc", None)
                nc = _CACHE.setdefault("nc", build_attention_nc())

    outs = []
    for b in range(2):
        acc = res.results[b * 4]["out"].astype(np.float32).copy()
        for g in range(1, 4):
            acc += res.results[b * 4 + g]["out"]
        outs.append(acc)
    return (np.stack(outs) + b_proj.astype(np.float32)).astype(np.float32)



# revision 6
# speedup vs baseline: 1.1199x; 1.1199x over previous
"""Multi-head attention block (B=2, N=2048, C=1024, H=16, D=64) on 8
Trainium2 NeuronCores — fused-pipeline version.

Sharding: core c -> batch b = c//4, head-group g = c%4 (tensor-parallel over
heads within a batch, 4 heads per core). QKV weights column-sharded, w_proj
row-sharded; each core emits a partial [N, C] projection which the host sums
per batch and then adds b_proj.

Final version (v4a):
- bf16 pipeline end to end (inputs pre-cast on host), tol 2e-2 passes at ~3e-3.
- Single fused emission: K(all) -> V(all) -> Q(0), then per-q-slab attention
  with Q(s+1)/proj(s-1) units interleaved between head-pairs so ACT exp and
  DVE work overlap the QKV/proj matmuls.
- Head-pair QK^T row-packing: the two heads of a pair sit at partitions
  0-63 / 64-127, so their K=64-contraction QK^T matmuls carry disjoint PE
  row-groups (tile_position auto-derived from base_partition) and execute
  concurrently on the 128x128 array; one exp instruction covers both heads'
  S^T chunk ([128, 1024]); PV lags one chunk.
- x shipped host-swizzled as [128, slab, kc, 512] so each per-slab DMA is a
  contiguous 8KB run per partition.
- PSUM: 2 S^T rings (2 banks each) + 2 po + 2 shared qkv/proj banks.
"""

import sys

sys.path.insert(0, "/opt/trn_rl_repo")

import numpy as np

from contextlib import ExitStack

import concourse.bacc as bacc
import concourse.tile as tile
from concourse import mybir

F32 = mybir.dt.float32
BF16 = mybir.dt.bfloat16

N = 2048
C = 1024
HL = 4  # heads per core
D = 64
KC = C // 128  # 8 contraction chunks
NS = N // 512  # 4 n-supers / slabs
MC = N // 128  # 16 m-chunks


def build_attention_nc(mm_dtype=BF16, loop_iters=None, stag=False, nring=2, glen=2, probes=frozenset()):
    # cascade probes: skipping a stage also skips consumers of its outputs
    probes = set(probes)
    if "qkt" in probes:
        probes |= {"exp"}
    if "exp" in probes:
        probes |= {"pv"}
    if "pv" in probes:
        probes |= {"recip"}
    if "recip" in probes:
        probes |= {"proj"}

    nc = bacc.Bacc(None, target_bir_lowering=False, debug=False)

    MMDT = mm_dtype
    xt = nc.dram_tensor("xt", [128, NS, KC, 512], MMDT, kind="ExternalInput")
    wq = nc.dram_tensor("wq", [C, 2, 128], MMDT, kind="ExternalInput")
    wk = nc.dram_tensor("wk", [C, 2, 128], MMDT, kind="ExternalInput")
    wv = nc.dram_tensor("wv", [C, 256], MMDT, kind="ExternalInput")
    bq = nc.dram_tensor("bq", [2, 128], F32, kind="ExternalInput")
    bk = nc.dram_tensor("bk", [2, 128], F32, kind="ExternalInput")
    bv = nc.dram_tensor("bv", [256], F32, kind="ExternalInput")
    wp = nc.dram_tensor("wp", [256, C], MMDT, kind="ExternalInput")
    onesv = nc.dram_tensor("onesv", [128, 64], MMDT, kind="ExternalInput")
    out = nc.dram_tensor("out", [N, C], MMDT, kind="ExternalOutput")

    with (
        tile.TileContext(nc) as tc,
        ExitStack() as ctx,
        nc.allow_low_precision(reason="bf16 matmul pipeline, tol 2e-2"),
    ):
        const = ctx.enter_context(tc.tile_pool(name="const", bufs=1))
        persist = ctx.enter_context(tc.tile_pool(name="persist", bufs=1))
        xt_pool = ctx.enter_context(tc.tile_pool(name="xt_pool", bufs=1))
        mm_ps = ctx.enter_context(tc.tile_pool(name="mm_ps", bufs=2, space="PSUM"))
        st_ps = ctx.enter_context(tc.tile_pool(name="st_ps", bufs=1, space="PSUM"))
        o_ps = ctx.enter_context(tc.tile_pool(name="o_ps", bufs=1, space="PSUM"))
        p_pool = ctx.enter_context(tc.tile_pool(name="p_pool", bufs=4))
        r_pool = ctx.enter_context(tc.tile_pool(name="r_pool", bufs=4))
        out_pool = ctx.enter_context(tc.tile_pool(name="out_pool", bufs=3))

        # --- constants / weights (emission order = DMA priority) ---
        wk_sb = const.tile([128, KC, 256], MMDT)
        wv_sb = const.tile([128, KC, 256], MMDT)
        wq_sb = const.tile([128, KC, 256], MMDT)
        nc.sync.dma_start(out=wk_sb, in_=wk.rearrange("(kc p) j m -> p kc (j m)", p=128))
        bk_sb = const.tile([128, 2], F32)
        nc.sync.dma_start(out=bk_sb, in_=bk.rearrange("j p -> p j"))
        nc.sync.dma_start(out=wv_sb, in_=wv.rearrange("(kc p) m -> p kc m", p=128))
        bv_rep = const.tile([128, 256], F32)
        nc.sync.dma_start(out=bv_rep, in_=bv[:].unsqueeze(0).partition_broadcast(128))
        nc.sync.dma_start(out=wq_sb, in_=wq.rearrange("(kc p) j m -> p kc (j m)", p=128))
        bq_sb = const.tile([128, 2], F32)
        nc.sync.dma_start(out=bq_sb, in_=bq.rearrange("j p -> p j"))
        wp_sb = const.tile([128, 2, C], MMDT)
        nc.sync.dma_start(out=wp_sb, in_=wp.rearrange("(kc p) n -> p kc n", p=128))

        # --- persistent intermediates ---
        qt_sb = persist.tile([128, 2, N], MMDT)  # [d(2 heads), pair, n]
        kt_sb = persist.tile([128, 2, N], MMDT)
        v_sb = persist.tile([128, MC, HL, 65], MMDT)  # [m, mc, head, d|1]
        ot_sb = persist.tile([128, 2, N], MMDT)  # [d(2 heads), hd-chunk, n]
        nc.sync.dma_start(
            out=v_sb[:, :, :, 64:65],
            in_=onesv.rearrange("p (a b c) -> p a b c", a=MC, b=HL, c=1),
        )

        def body():
            # x resident for all slabs, loaded per-slab for fine-grain deps
            xts = xt_pool.tile([128, NS, KC, 512], MMDT, tag="xts")
            ptdummy = None
            if "ptconst" in probes:
                ptdummy = xt_pool.tile([128, 2, 512], MMDT, tag="ptdummy")
                nc.vector.memset(ptdummy, 0.01)
            if "xdma" not in probes:
                for s in range(NS):
                    nc.sync.dma_start(
                        out=xts[:, s, :, :],
                        in_=xt[:, s, :, :],
                    )

            def emit_qk(s, j, wsb, bsb, dst, nm):
                if "qkv" in probes:
                    return
                ps = mm_ps.tile([128, 512], F32, tag="mmps", name=f"{nm}{s}{j}")
                for kc in range(KC):
                    nc.tensor.matmul(
                        ps,
                        lhsT=wsb[:, kc, j * 128 : (j + 1) * 128],
                        rhs=xts[:, s, kc, :],
                        start=kc == 0,
                        stop=kc == KC - 1,
                    )
                nc.vector.tensor_scalar_add(
                    out=dst[:, j, s * 512 : (s + 1) * 512],
                    in0=ps,
                    scalar1=bsb[:, j : j + 1],
                )

            def emit_v(s, jj):
                if "qkv" in probes:
                    return
                ps = mm_ps.tile([128, 256], F32, tag="mmps", name=f"v{s}{jj}")
                for kc in range(KC):
                    nc.tensor.matmul(
                        ps,
                        lhsT=xts[:, s, kc, jj * 128 : (jj + 1) * 128],
                        rhs=wv_sb[:, kc, :],
                        start=kc == 0,
                        stop=kc == KC - 1,
                    )
                mc = s * 4 + jj
                nc.vector.tensor_add(
                    out=v_sb[:, mc, :, 0:64],
                    in0=ps.rearrange("p (h d) -> p h d", h=HL),
                    in1=bv_rep.rearrange("p (h d) -> p h d", h=HL),
                )

            def emit_proj(s, ntl):
                if "proj" in probes:
                    return
                nt = s * 4 + ntl
                for cc in range(2):
                    ps = mm_ps.tile([128, 512], F32, tag="mmps", name=f"pj{nt}{cc}")
                    for hdc in range(2):
                        nc.tensor.matmul(
                            ps,
                            lhsT=ot_sb[:, hdc, nt * 128 : (nt + 1) * 128],
                            rhs=wp_sb[:, hdc, cc * 512 : (cc + 1) * 512],
                            start=hdc == 0,
                            stop=hdc == 1,
                        )
                    so = out_pool.tile([128, 512], MMDT, tag="so")
                    nc.vector.tensor_copy(out=so, in_=ps)
                    nc.sync.dma_start(
                        out=out[nt * 128 : (nt + 1) * 128, cc * 512 : (cc + 1) * 512],
                        in_=so,
                    )

            # ---- prologue: K for all slabs (every m-chunk of pair (0,0)
            # reads K), V(0) (PV chunks 0-3), Q(0). V(1..3) are emitted as
            # per-chunk units inside pair (0,0) just ahead of their PV use;
            # Q(s+1)/proj(s-1) spread over later pairs. Keeping the prologue
            # minimal gets the first exp onto ACT (the serial bottleneck)
            # ~14 us earlier than emitting all of QKV up front. ----
            for s in range(NS):
                for j in range(2):
                    emit_qk(s, j, wk_sb, bk_sb, kt_sb, "k")
            for jj in range(4):
                emit_v(0, jj)
            for j in range(2):
                emit_qk(0, j, wq_sb, bq_sb, qt_sb, "q")

            # per-(s,j) units keyed by chunk slot: emitted between exp(mc)
            # and PV(mc-1) so the next QK^T is never delayed by more than
            # one unit (ACT stays fed).
            def qk_unit(s2, j2):
                return lambda: emit_qk(s2, j2, wq_sb, bq_sb, qt_sb, "q")

            U = {
                (0, 0): {
                    slot: (lambda vs=1 + slot // 4, jj=slot % 4: emit_v(vs, jj))
                    for slot in range(12)
                },
                (0, 1): {2: qk_unit(1, 0), 8: qk_unit(1, 1)},
                (1, 0): {4 * n + 2: (lambda n=n: emit_proj(0, n)) for n in range(4)},
                (1, 1): {2: qk_unit(2, 0), 8: qk_unit(2, 1)},
                (2, 0): {4 * n + 2: (lambda n=n: emit_proj(1, n)) for n in range(4)},
                (2, 1): {2: qk_unit(3, 0), 8: qk_unit(3, 1)},
                (3, 0): {4 * n + 2: (lambda n=n: emit_proj(2, n)) for n in range(4)},
                (3, 1): {},
            }

            # rings hold one m-chunk for BOTH heads of a pair: [128, hh, 512]
            rings = [
                st_ps.tile([128, 2, 512], F32, tag=f"ring{u}", name=f"ring{u}")
                for u in range(nring)
            ]

            for s in range(NS):
                for j in range(2):
                    units = U[(s, j)]
                    po2 = [
                        o_ps.tile([128, 512], F32, tag=f"po{hh}", name=f"po_s{s}j{j}h{hh}")
                        for hh in range(2)
                    ]

                    def emit_pv(prev):
                        if "pv" in probes:
                            return
                        pt, mc = prev
                        if ptdummy is not None:
                            pt = ptdummy
                        for hh in range(2):
                            nc.tensor.matmul(
                                po2[hh][0:65, :],
                                lhsT=v_sb[:, mc, 2 * j + hh, :],
                                rhs=pt[:, hh, :],
                                start=mc == 0,
                                stop=mc == MC - 1,
                            )

                    prev = None
                    for mc in range(MC):
                        ring = rings[mc % nring]
                        # both heads' QK^T back-to-back: base partitions 0/64
                        # -> disjoint PE row groups, run concurrently
                        for hh in range(2 if "qkt" not in probes else 0):
                            base = hh * 64
                            nc.tensor.matmul(
                                ring[:, hh, :],
                                lhsT=kt_sb[
                                    base : base + 64, j, mc * 128 : (mc + 1) * 128
                                ],
                                rhs=qt_sb[
                                    base : base + 64, j, s * 512 : (s + 1) * 512
                                ],
                                start=True,
                                stop=True,
                            )
                        pt = p_pool.tile([128, 2, 512], MMDT, tag="pt")
                        if "exp" not in probes:
                            nc.scalar.activation(
                                out=pt,
                                in_=ring,
                                func=mybir.ActivationFunctionType.Exp,
                            )
                        if mc in units:
                            units[mc]()
                        if prev is not None:
                            emit_pv(prev)
                        prev = (pt, mc)
                    emit_pv(prev)

                    # Copy the accumulators out of PSUM immediately: the
                    # recip->broadcast->mul chain (5 cross-engine hops) then
                    # runs off the critical path, and po2's banks are free
                    # for the next pair's PV after just two quick DVE copies
                    # (HW ablation: the in-PSUM tail cost +73us/iter).
                    for hh in range(2 if "recip" not in probes else 0):
                        po_sb = r_pool.tile(
                            [65, 512], F32, tag=f"po_sb{hh}", name=f"posb_s{s}j{j}h{hh}"
                        )
                        nc.vector.tensor_copy(out=po_sb, in_=po2[hh][0:65, :])
                        recip = r_pool.tile([1, 512], F32, tag="recip")
                        nc.vector.reciprocal(out=recip, in_=po_sb[64:65, :])
                        rden_sb = r_pool.tile(
                            [64, 512], F32, tag="rden_sb", name=f"rd_s{s}j{j}h{hh}"
                        )
                        nc.gpsimd.partition_broadcast(rden_sb, recip)
                        nc.vector.tensor_mul(
                            out=ot_sb[
                                hh * 64 : hh * 64 + 64,
                                j,
                                s * 512 : (s + 1) * 512,
                            ],
                            in0=po_sb[0:64, :],
                            in1=rden_sb,
                        )

            # epilogue: proj of last slab
            for n in range(4):
                emit_proj(3, n)

        if loop_iters is None:
            body()
        else:
            with tc.For_i(0, loop_iters, 1, staggered_reset=stag):
                body()

    nc.compile()
    return nc


EMBED_DIM = 1024
NUM_HEADS = 16
HEAD_DIM = 64
HPC = 4

_CACHE = {}


def _bf16(a):
    import ml_dtypes

    return np.asarray(a, dtype=ml_dtypes.bfloat16)


def _make_in_maps(x, w_qkv, b_qkv, w_proj):
    scale = HEAD_DIM ** -0.5
    def _swizzle(xb):
        xt = np.ascontiguousarray(xb.T)  # [C, N]
        return _bf16(
            np.ascontiguousarray(
                xt.reshape(KC, 128, NS, 512).transpose(1, 2, 0, 3)
            )
        )

    xts = [_swizzle(x[b]) for b in range(2)]
    ones = _bf16(np.ones((128, 64), np.float32))
    in_maps = []
    for core in range(8):
        b, g = core // 4, core % 4
        cols = slice(g * HPC * HEAD_DIM, (g + 1) * HPC * HEAD_DIM)
        wq = (w_qkv[:, 0:C][:, cols] * scale).astype(np.float32)
        wk = w_qkv[:, C : 2 * C][:, cols].astype(np.float32)
        wv = w_qkv[:, 2 * C : 3 * C][:, cols].astype(np.float32)
        bq = (b_qkv[0:C][cols] * scale).astype(np.float32)
        bk = b_qkv[C : 2 * C][cols].astype(np.float32)
        bvv = b_qkv[2 * C : 3 * C][cols].astype(np.float32)
        wpm = np.ascontiguousarray(w_proj[cols.start : cols.stop, :]).astype(np.float32)
        in_maps.append(
            {
                "xt": xts[b],
                "wq": _bf16(np.ascontiguousarray(wq.reshape(C, 2, 128))),
                "wk": _bf16(np.ascontiguousarray(wk.reshape(C, 2, 128))),
                "wv": _bf16(np.ascontiguousarray(wv)),
                "bq": np.ascontiguousarray(bq.reshape(2, 128)),
                "bk": np.ascontiguousarray(bk.reshape(2, 128)),
                "bv": np.ascontiguousarray(bvv),
                "wp": _bf16(wpm),
                "onesv": ones,
            }
        )
    return in_maps


def kernel(x, w_qkv, b_qkv, w_proj, b_proj):
    from concourse.bass_utils import run_bass_kernel_spmd

    x = np.asarray(x)
    w_qkv = np.asarray(w_qkv)
    b_qkv = np.asarray(b_qkv)
    w_proj = np.asarray(w_proj)
    b_proj = np.asarray(b_proj)

    if "nc" not in _CACHE:
        _CACHE["nc"] = build_attention_nc()
    nc = _CACHE["nc"]

    in_maps = _make_in_maps(x, w_qkv, b_qkv, w_proj)
    # The device transiently wedges on some runs (INTERNAL error on a NEFF
    # that runs clean on retry). Retry, then rebuild once, before giving up.
    import os

    res = None
    for attempt in range(3):
        try:
            res = run_bass_kernel_spmd(nc, in_maps, core_ids=list(range(8)))
            break
        except Exception:
            if attempt == 2:
                raise
            os.environ.setdefault("NEURON_RT_RESET_CORES", "1")
            if attempt == 1:
                _CACHE.pop("nc", None)
                nc = _CACHE.setdefault("nc", build_attention_nc())

    outs = []
    for b in range(2):
        acc = res.results[b * 4]["out"].astype(np.float32).copy()
        for g in range(1, 4):
            acc += res.results[b * 4 + g]["out"]
        outs.append(acc)
    return (np.stack(outs) + b_proj.astype(np.float32)).astype(np.float32)



# revision 8
# speedup vs baseline: 1.1291x; 1.0082x over previous
"""Multi-head attention block (B=2, N=2048, C=1024, H=16, D=64) on 8
Trainium2 NeuronCores — fused-pipeline version.

Sharding: core c -> batch b = c//4, head-group g = c%4 (tensor-parallel over
heads within a batch, 4 heads per core). QKV weights column-sharded, w_proj
row-sharded; each core emits a partial [N, C] projection which the host sums
per batch and then adds b_proj.

Version v9 (this session; baseline v4a was 255-282us, v9 measures ~200-254us
per-iteration by interleaved loop-delta):
- bf16 pipeline end to end (inputs pre-cast on host), tol 2e-2 passes at ~3e-3.
- Minimal prologue (K all slabs, V(0), Q(0)); V(1..3)/Q(s+1)/proj(s-1) are
  emitted as per-chunk units inside the attention loops so the ACT exp stream
  (the serial bottleneck, ~1.04us per [128,1024] chunk) stays fed.
- Head-pair QK^T row-packing: the two heads of a pair sit at partitions
  0-63 / 64-127, so their K=64-contraction QK^T matmuls carry disjoint PE
  row-groups (tile_position auto-derived from base_partition) and execute
  concurrently on the 128x128 array; one exp instruction covers both heads'
  S^T chunk ([128, 1024]); PV lags one chunk.
- po accumulators are copied PSUM->SBUF right after the last PV so the
  recip->broadcast->mul normalization chain runs off the critical path and
  the po PSUM banks free early (HW ablation: in-PSUM tail cost +73us/iter).
- 2x-unrolled For_i with parity-alternating xts/kt/qt/v/ot buffers: iteration
  i+1's x DMA (on the ACT hwdge queue, so not stuck behind out-stores) and
  QKV emission overlap iteration i's attention tail (-60us/iter).
- x shipped host-swizzled as [128, slab, kc, 512] so each per-slab DMA is a
  contiguous 8KB run per partition.
- PSUM: 2 S^T rings (2 banks each) + 2 po + 2 shared qkv/proj banks.
"""

import sys

sys.path.insert(0, "/opt/trn_rl_repo")

import numpy as np

from contextlib import ExitStack

import concourse.bacc as bacc
import concourse.tile as tile
from concourse import mybir

F32 = mybir.dt.float32
BF16 = mybir.dt.bfloat16

N = 2048
C = 1024
HL = 4  # heads per core
D = 64
KC = C // 128  # 8 contraction chunks
NS = N // 512  # 4 n-supers / slabs
MC = N // 128  # 16 m-chunks


def build_attention_nc(mm_dtype=BF16, loop_iters=None, stag=False, nring=2, glen=2, probes=frozenset()):
    # cascade probes: skipping a stage also skips consumers of its outputs
    probes = set(probes)
    if "qkt" in probes:
        probes |= {"exp"}
    if "exp" in probes:
        probes |= {"pv"}
    if "pv" in probes:
        probes |= {"recip"}
    if "recip" in probes:
        probes |= {"proj"}

    nc = bacc.Bacc(None, target_bir_lowering=False, debug=False)

    MMDT = mm_dtype
    xt = nc.dram_tensor("xt", [128, NS, KC, 512], MMDT, kind="ExternalInput")
    wq = nc.dram_tensor("wq", [C, 2, 128], MMDT, kind="ExternalInput")
    wk = nc.dram_tensor("wk", [C, 2, 128], MMDT, kind="ExternalInput")
    wv = nc.dram_tensor("wv", [C, 256], MMDT, kind="ExternalInput")
    bq = nc.dram_tensor("bq", [2, 128], F32, kind="ExternalInput")
    bk = nc.dram_tensor("bk", [2, 128], F32, kind="ExternalInput")
    bv = nc.dram_tensor("bv", [256], F32, kind="ExternalInput")
    wp = nc.dram_tensor("wp", [256, C], MMDT, kind="ExternalInput")
    onesv = nc.dram_tensor("onesv", [128, 64], MMDT, kind="ExternalInput")
    out = nc.dram_tensor("out", [N, C], MMDT, kind="ExternalOutput")

    with (
        tile.TileContext(nc) as tc,
        ExitStack() as ctx,
        nc.allow_low_precision(reason="bf16 matmul pipeline, tol 2e-2"),
    ):
        const = ctx.enter_context(tc.tile_pool(name="const", bufs=1))
        persist = ctx.enter_context(tc.tile_pool(name="persist", bufs=1))
        xt_pool = ctx.enter_context(tc.tile_pool(name="xt_pool", bufs=1))
        mm_ps = ctx.enter_context(tc.tile_pool(name="mm_ps", bufs=2, space="PSUM"))
        st_ps = ctx.enter_context(tc.tile_pool(name="st_ps", bufs=1, space="PSUM"))
        o_ps = ctx.enter_context(tc.tile_pool(name="o_ps", bufs=1, space="PSUM"))
        p_pool = ctx.enter_context(tc.tile_pool(name="p_pool", bufs=4))
        r_pool = ctx.enter_context(tc.tile_pool(name="r_pool", bufs=4))
        out_pool = ctx.enter_context(tc.tile_pool(name="out_pool", bufs=3))

        # --- constants / weights (emission order = DMA priority) ---
        wk_sb = const.tile([128, KC, 256], MMDT)
        wv_sb = const.tile([128, KC, 256], MMDT)
        wq_sb = const.tile([128, KC, 256], MMDT)
        nc.sync.dma_start(out=wk_sb, in_=wk.rearrange("(kc p) j m -> p kc (j m)", p=128))
        bk_sb = const.tile([128, 2], F32)
        nc.sync.dma_start(out=bk_sb, in_=bk.rearrange("j p -> p j"))
        nc.sync.dma_start(out=wv_sb, in_=wv.rearrange("(kc p) m -> p kc m", p=128))
        bv_rep = const.tile([128, 256], F32)
        nc.sync.dma_start(out=bv_rep, in_=bv[:].unsqueeze(0).partition_broadcast(128))
        nc.sync.dma_start(out=wq_sb, in_=wq.rearrange("(kc p) j m -> p kc (j m)", p=128))
        bq_sb = const.tile([128, 2], F32)
        nc.sync.dma_start(out=bq_sb, in_=bq.rearrange("j p -> p j"))
        wp_sb = const.tile([128, 2, C], MMDT)
        nc.sync.dma_start(out=wp_sb, in_=wp.rearrange("(kc p) n -> p kc n", p=128))

        # --- persistent intermediates, double-buffered by loop parity so
        # iteration i+1's QKV emission + x DMA can overlap iteration i's
        # attention tail (the For_i body is unrolled 2x) ---
        qt_sbs = [persist.tile([128, 2, N], MMDT, name=f"qt{p}") for p in range(2)]
        kt_sbs = [persist.tile([128, 2, N], MMDT, name=f"kt{p}") for p in range(2)]
        v_sbs = [persist.tile([128, MC, HL, 65], MMDT, name=f"v{p}") for p in range(2)]
        ot_sbs = [persist.tile([128, 2, N], MMDT, name=f"ot{p}") for p in range(2)]
        for p in range(2):
            nc.sync.dma_start(
                out=v_sbs[p][:, :, :, 64:65],
                in_=onesv.rearrange("p (a b c) -> p a b c", a=MC, b=HL, c=1),
            )

        def body(par):
            qt_sb, kt_sb = qt_sbs[par], kt_sbs[par]
            v_sb, ot_sb = v_sbs[par], ot_sbs[par]
            # x resident for all slabs, loaded per-slab for fine-grain deps.
            # Issued on the ACT hwdge queue so these loads are not stuck
            # behind the previous iteration's out-stores on the SP queue.
            xts = xt_pool.tile([128, NS, KC, 512], MMDT, tag=f"xts{par}")
            ptdummy = None
            if "ptconst" in probes:
                ptdummy = xt_pool.tile([128, 2, 512], MMDT, tag="ptdummy")
                nc.vector.memset(ptdummy, 0.01)
            if "xdma" not in probes:
                for s in range(NS):
                    nc.scalar.dma_start(
                        out=xts[:, s, :, :],
                        in_=xt[:, s, :, :],
                    )

            def emit_qk(s, j, wsb, bsb, dst, nm):
                if "qkv" in probes:
                    return
                ps = mm_ps.tile([128, 512], F32, tag="mmps", name=f"{nm}{s}{j}")
                for kc in range(KC):
                    nc.tensor.matmul(
                        ps,
                        lhsT=wsb[:, kc, j * 128 : (j + 1) * 128],
                        rhs=xts[:, s, kc, :],
                        start=kc == 0,
                        stop=kc == KC - 1,
                    )
                nc.vector.tensor_scalar_add(
                    out=dst[:, j, s * 512 : (s + 1) * 512],
                    in0=ps,
                    scalar1=bsb[:, j : j + 1],
                )

            def emit_v(s, jj):
                if "qkv" in probes:
                    return
                ps = mm_ps.tile([128, 256], F32, tag="mmps", name=f"v{s}{jj}")
                for kc in range(KC):
                    nc.tensor.matmul(
                        ps,
                        lhsT=xts[:, s, kc, jj * 128 : (jj + 1) * 128],
                        rhs=wv_sb[:, kc, :],
                        start=kc == 0,
                        stop=kc == KC - 1,
                    )
                mc = s * 4 + jj
                nc.vector.tensor_add(
                    out=v_sb[:, mc, :, 0:64],
                    in0=ps.rearrange("p (h d) -> p h d", h=HL),
                    in1=bv_rep.rearrange("p (h d) -> p h d", h=HL),
                )

            def emit_proj(s, ntl):
                if "proj" in probes:
                    return
                nt = s * 4 + ntl
                for cc in range(2):
                    ps = mm_ps.tile([128, 512], F32, tag="mmps", name=f"pj{nt}{cc}")
                    for hdc in range(2):
                        nc.tensor.matmul(
                            ps,
                            lhsT=ot_sb[:, hdc, nt * 128 : (nt + 1) * 128],
                            rhs=wp_sb[:, hdc, cc * 512 : (cc + 1) * 512],
                            start=hdc == 0,
                            stop=hdc == 1,
                        )
                    so = out_pool.tile([128, 512], MMDT, tag="so")
                    nc.vector.tensor_copy(out=so, in_=ps)
                    nc.sync.dma_start(
                        out=out[nt * 128 : (nt + 1) * 128, cc * 512 : (cc + 1) * 512],
                        in_=so,
                    )

            # ---- prologue: K for all slabs (every m-chunk of pair (0,0)
            # reads K), V(0) (PV chunks 0-3), Q(0). V(1..3) are emitted as
            # per-chunk units inside pair (0,0) just ahead of their PV use;
            # Q(s+1)/proj(s-1) spread over later pairs. Keeping the prologue
            # minimal gets the first exp onto ACT (the serial bottleneck)
            # ~14 us earlier than emitting all of QKV up front. ----
            for s in range(NS):
                for j in range(2):
                    emit_qk(s, j, wk_sb, bk_sb, kt_sb, "k")
            for jj in range(4):
                emit_v(0, jj)
            for j in range(2):
                emit_qk(0, j, wq_sb, bq_sb, qt_sb, "q")

            # per-(s,j) units keyed by chunk slot: emitted between exp(mc)
            # and PV(mc-1) so the next QK^T is never delayed by more than
            # one unit (ACT stays fed).
            def qk_unit(s2, j2):
                return lambda: emit_qk(s2, j2, wq_sb, bq_sb, qt_sb, "q")

            U = {
                (0, 0): {
                    slot: (lambda vs=1 + slot // 4, jj=slot % 4: emit_v(vs, jj))
                    for slot in range(12)
                },
                (0, 1): {2: qk_unit(1, 0), 8: qk_unit(1, 1)},
                (1, 0): {4 * n + 2: (lambda n=n: emit_proj(0, n)) for n in range(4)},
                (1, 1): {2: qk_unit(2, 0), 8: qk_unit(2, 1)},
                (2, 0): {4 * n + 2: (lambda n=n: emit_proj(1, n)) for n in range(4)},
                (2, 1): {2: qk_unit(3, 0), 8: qk_unit(3, 1)},
                (3, 0): {4 * n + 2: (lambda n=n: emit_proj(2, n)) for n in range(4)},
                (3, 1): {},
            }

            # rings hold one m-chunk for BOTH heads of a pair: [128, hh, 512]
            rings = [
                st_ps.tile([128, 2, 512], F32, tag=f"ring{u}", name=f"ring{u}")
                for u in range(nring)
            ]

            for s in range(NS):
                for j in range(2):
                    units = U[(s, j)]
                    po2 = [
                        o_ps.tile([128, 512], F32, tag=f"po{hh}", name=f"po_s{s}j{j}h{hh}")
                        for hh in range(2)
                    ]

                    def emit_pv(prev):
                        if "pv" in probes:
                            return
                        pt, mc = prev
                        if ptdummy is not None:
                            pt = ptdummy
                        for hh in range(2):
                            nc.tensor.matmul(
                                po2[hh][0:65, :],
                                lhsT=v_sb[:, mc, 2 * j + hh, :],
                                rhs=pt[:, hh, :],
                                start=mc == 0,
                                stop=mc == MC - 1,
                            )

                    prev = None
                    for mc in range(MC):
                        ring = rings[mc % nring]
                        # both heads' QK^T back-to-back: base partitions 0/64
                        # -> disjoint PE row groups, run concurrently
                        for hh in range(2 if "qkt" not in probes else 0):
                            base = hh * 64
                            nc.tensor.matmul(
                                ring[:, hh, :],
                                lhsT=kt_sb[
                                    base : base + 64, j, mc * 128 : (mc + 1) * 128
                                ],
                                rhs=qt_sb[
                                    base : base + 64, j, s * 512 : (s + 1) * 512
                                ],
                                start=True,
                                stop=True,
                            )
                        pt = p_pool.tile([128, 2, 512], MMDT, tag="pt")
                        if "exp" not in probes:
                            nc.scalar.activation(
                                out=pt,
                                in_=ring,
                                func=mybir.ActivationFunctionType.Exp,
                            )
                        if mc in units:
                            units[mc]()
                        if prev is not None:
                            emit_pv(prev)
                        prev = (pt, mc)
                    emit_pv(prev)

                    # Copy the accumulators out of PSUM immediately: the
                    # recip->broadcast->mul chain (5 cross-engine hops) then
                    # runs off the critical path, and po2's banks are free
                    # for the next pair's PV after just two quick DVE copies
                    # (HW ablation: the in-PSUM tail cost +73us/iter).
                    for hh in range(2 if "recip" not in probes else 0):
                        po_sb = r_pool.tile(
                            [65, 512], F32, tag=f"po_sb{hh}", name=f"posb_s{s}j{j}h{hh}"
                        )
                        nc.vector.tensor_copy(out=po_sb, in_=po2[hh][0:65, :])
                        recip = r_pool.tile([1, 512], F32, tag="recip")
                        nc.vector.reciprocal(out=recip, in_=po_sb[64:65, :])
                        rden_sb = r_pool.tile(
                            [64, 512], F32, tag="rden_sb", name=f"rd_s{s}j{j}h{hh}"
                        )
                        nc.gpsimd.partition_broadcast(rden_sb, recip)
                        nc.vector.tensor_mul(
                            out=ot_sb[
                                hh * 64 : hh * 64 + 64,
                                j,
                                s * 512 : (s + 1) * 512,
                            ],
                            in0=po_sb[0:64, :],
                            in1=rden_sb,
                        )

            # epilogue: proj of last slab
            for n in range(4):
                emit_proj(3, n)

        if loop_iters is None:
            body(0)
        else:
            # 2x-unrolled loop with parity-alternating buffers: iteration
            # boundaries pipeline (next x DMA + QKV emission vs current
            # attention tail). Odd remainder handled by a trailing body.
            with tc.For_i(0, loop_iters // 2, 1, staggered_reset=stag):
                body(0)
                body(1)
            if loop_iters % 2:
                body(0)

    nc.compile()
    return nc


EMBED_DIM = 1024
NUM_HEADS = 16
HEAD_DIM = 64
HPC = 4

_CACHE = {}


def _bf16(a):
    import ml_dtypes

    return np.asarray(a, dtype=ml_dtypes.bfloat16)


def _make_in_maps(x, w_qkv, b_qkv, w_proj):
    scale = HEAD_DIM ** -0.5
    def _swizzle(xb):
        xt = np.ascontiguousarray(xb.T)  # [C, N]
        return _bf16(
            np.ascontiguousarray(
                xt.reshape(KC, 128, NS, 512).transpose(1, 2, 0, 3)
            )
        )

    xts = [_swizzle(x[b]) for b in range(2)]
    ones = _bf16(np.ones((128, 64), np.float32))
    in_maps = []
    for core in range(8):
        b, g = core // 4, core % 4
        cols = slice(g * HPC * HEAD_DIM, (g + 1) * HPC * HEAD_DIM)
        wq = (w_qkv[:, 0:C][:, cols] * scale).astype(np.float32)
        wk = w_qkv[:, C : 2 * C][:, cols].astype(np.float32)
        wv = w_qkv[:, 2 * C : 3 * C][:, cols].astype(np.float32)
        bq = (b_qkv[0:C][cols] * scale).astype(np.float32)
        bk = b_qkv[C : 2 * C][cols].astype(np.float32)
        bvv = b_qkv[2 * C : 3 * C][cols].astype(np.float32)
        wpm = np.ascontiguousarray(w_proj[cols.start : cols.stop, :]).astype(np.float32)
        in_maps.append(
            {
                "xt": xts[b],
                "wq": _bf16(np.ascontiguousarray(wq.reshape(C, 2, 128))),
                "wk": _bf16(np.ascontiguousarray(wk.reshape(C, 2, 128))),
                "wv": _bf16(np.ascontiguousarray(wv)),
                "bq": np.ascontiguousarray(bq.reshape(2, 128)),
                "bk": np.ascontiguousarray(bk.reshape(2, 128)),
                "bv": np.ascontiguousarray(bvv),
                "wp": _bf16(wpm),
                "onesv": ones,
            }
        )
    return in_maps


def kernel(x, w_qkv, b_qkv, w_proj, b_proj):
    from concourse.bass_utils import run_bass_kernel_spmd

    x = np.asarray(x)
    w_qkv = np.asarray(w_qkv)
    b_qkv = np.asarray(b_qkv)
    w_proj = np.asarray(w_proj)
    b_proj = np.asarray(b_proj)

    if "nc" not in _CACHE:
        _CACHE["nc"] = build_attention_nc()
    nc = _CACHE["nc"]

    in_maps = _make_in_maps(x, w_qkv, b_qkv, w_proj)
    # The device transiently wedges on some runs (INTERNAL error on a NEFF
    # that runs clean on retry). Retry, then rebuild once, before giving up.
    import os

    res = None
    for attempt in range(3):
        try:
            res = run_bass_kernel_spmd(nc, in_maps, core_ids=list(range(8)))
            break
        except Exception:
            if attempt == 2:
                raise
            os.environ.setdefault("NEURON_RT_RESET_CORES", "1")
            if attempt == 1:
                _CACHE.pop("nc", None)
                nc = _CACHE.setdefault("nc", build_attention_nc())

    outs = []
    for b in range(2):
        acc = res.results[b * 4]["out"].astype(np.float32).copy()
        for g in range(1, 4):
            acc += res.results[b * 4 + g]["out"]
        outs.append(acc)
    return (np.stack(outs) + b_proj.astype(np.float32)).astype(np.float32)

